# revision 6
# baseline (speedup 1.0000x reference)
"""BitLinear TRN2 kernel: y = x @ W(pweight,nweight)^T + bias.

Sharding: 8 cores = 4 token-shards x 2 out-feature-shards.
Per core: xT_c [2048, 4096] (token slice, uploaded transposed as part of the
sharding layout), pw/nw [1024, 2048, 4] (out-feature slice).

Device pipeline (bf16 compute, fp32 PSUM accumulation):
  weights: DMA pw/nw fp32 -> ACT sigmoid (bf16 out) -> DVE subtract
           -> PE transpose (128x128 blocks) -> PE combine-matmul with a
           [128,32] constant C (C[4i+n, i] = exps[n]*sigmoid(mask[n])*scale)
           -> wT [i, o] bf16 in SBUF
  x:       SWDGE DMA-cast fp32->bf16 of transposed slabs -> xT [i, t] bf16
  main:    psum[t,o] += xT_tile.T @ wT_tile over 16 i-tiles; DVE adds bias
           (host-replicated [128, OC] tile) during PSUM->SBUF copy; DMA out.

bias path: bit_ste is an exact identity on the reference's bias_raw values
(k/15 grid), computed host-side along with the tiny C matrix.
"""

import numpy as np

import concourse.bass as bass
import concourse.mybir as mybir
import concourse.tile as tile
from concourse import bacc
from concourse.bass_utils import run_bass_kernel_spmd
from concourse.masks import make_identity

N_CORES = 8
R, C = 4, 2  # token shards x out-feature shards
T, I, O, NB = 16384, 2048, 2048, 4
TQ, OC = T // R, O // C  # 4096 tokens, 1024 outs per core
P = 128
IN = I * NB  # 8192 flattened (i, n) columns of pw/nw
WCH = 2048  # weight free-chunk: 512 i x 4 n
N_IT = I // P  # 16 i-tiles
N_TT = TQ // P  # 32 t-tiles
N_OB = OC // P  # 8 o-blocks
OSH = OC // R  # 256: o-rows of weight prep done locally per core
N_OBP = OSH // P  # 2 local o-blocks
N_WCH = IN // WCH  # 4 chunks per o-block
TSLAB = 512  # tokens per x slab (4 t-tiles)
N_SLAB = TQ // TSLAB
DT = mybir.dt.bfloat16

_BUILT = None


def _build_bass(reps=1, mode='full'):
    nc = bacc.Bacc("TRN2", debug=False, num_devices=N_CORES)

    xt_d = nc.dram_tensor("xt", [I, TQ], mybir.dt.float32, kind="ExternalInput").ap()
    pw_d = nc.dram_tensor("pw", [OSH, IN], mybir.dt.float32, kind="ExternalInput").ap()
    nw_d = nc.dram_tensor("nw", [OSH, IN], mybir.dt.float32, kind="ExternalInput").ap()
    cv_d = nc.dram_tensor("cvec", [P, NB], mybir.dt.bfloat16, kind="ExternalInput").ap()
    bias_d = nc.dram_tensor("bias", [P, OC], mybir.dt.float32, kind="ExternalInput").ap()
    y_d = nc.dram_tensor("y", [TQ, OC], mybir.dt.float32, kind="ExternalOutput").ap()

    with tile.TileContext(nc) as tc:
        with (
            tc.tile_pool(name="const", bufs=1) as const_pool,
            tc.tile_pool(name="wT", bufs=2) as wT_pool,
            tc.tile_pool(name="wpart", bufs=2) as wpart_pool,
            tc.tile_pool(name="dram", bufs=2, space="DRAM") as dram_pool,
            tc.tile_pool(name="wio", bufs=2) as wio_pool,
            tc.tile_pool(name="sig", bufs=2) as sig_pool,
            tc.tile_pool(name="soft", bufs=2) as soft_pool,
            tc.tile_pool(name="scl", bufs=2) as scl_pool,
            tc.tile_pool(name="wn", bufs=2) as wn_pool,
            tc.tile_pool(name="xs", bufs=2) as xs_pool,
            tc.tile_pool(name="yo", bufs=2) as yo_pool,
            tc.tile_pool(name="wtp_ps", bufs=2, space="PSUM") as wtp_ps,
            tc.tile_pool(name="mm_ps", bufs=2, space="PSUM") as mm_ps,
        ):
            ident = const_pool.tile([P, P], mybir.dt.float32)
            make_identity(nc, ident[:])
            cv_sb = const_pool.tile([P, NB], DT)
            nc.sync.dma_start(cv_sb[:], cv_d[:])
            bias_sb = const_pool.tile([P, OC], mybir.dt.float32)
            nc.sync.dma_start(bias_sb[:], bias_d[:])

            for _rep in range(reps):
                wT = wT_pool.tile([P, N_IT, OC], DT, tag="wT")
                wpart = wpart_pool.tile([P, N_IT, OSH], DT, tag="wpart")

                # ---------------- weight stage (local o-slice) ----------------
                for ob in (range(0) if mode == 'mm' else range(N_OBP)):
                    orow = slice(ob * P, (ob + 1) * P)
                    for ch in range(N_WCH):
                        fcol = slice(ch * WCH, (ch + 1) * WCH)
                        pwt = wio_pool.tile([P, WCH], mybir.dt.float32, tag="pw")
                        nc.sync.dma_start(pwt[:], pw_d[orow, fcol])
                        nwt = wio_pool.tile([P, WCH], mybir.dt.float32, tag="nw")
                        nc.sync.dma_start(nwt[:], nw_d[orow, fcol])

                        if mode == 'dma':
                            continue
                        sp = sig_pool.tile([P, WCH], DT, tag="sp")
                        nc.scalar.activation(
                            sp[:], pwt[:], mybir.ActivationFunctionType.Sigmoid
                        )
                        sn = sig_pool.tile([P, WCH], DT, tag="sn")
                        nc.scalar.activation(
                            sn[:], nwt[:], mybir.ActivationFunctionType.Sigmoid
                        )
                        soft = soft_pool.tile([P, WCH], DT, tag="soft")
                        nc.vector.tensor_sub(out=soft[:], in0=sp[:], in1=sn[:])

                        # scaled[o, i, n] = soft * c[n]; w_nat[o, i] = sum_n
                        ICH = WCH // NB  # 512 i per chunk = i-tiles 4ch..4ch+3
                        scaled = scl_pool.tile([P, WCH], DT, tag="scl")
                        nc.vector.tensor_tensor(
                            scaled[:].rearrange("p (i n) -> p i n", n=NB),
                            soft[:].rearrange("p (i n) -> p i n", n=NB),
                            cv_sb[:, None, :].to_broadcast((P, ICH, NB)),
                            mybir.AluOpType.mult,
                        )
                        wn = wn_pool.tile([P, ICH], mybir.dt.float32, tag="wn")
                        nc.vector.tensor_reduce(
                            wn[:],
                            scaled[:].rearrange("p (i n) -> p i n", n=NB),
                            axis=mybir.AxisListType.X,
                            op=mybir.AluOpType.add,
                        )
                        # transpose w_nat [o 128, i 512] -> wT [i, o] per 128-block
                        wtp = wtp_ps.tile([P, 512], mybir.dt.float32, tag="wtp")
                        for b in range(4):
                            nc.tensor.transpose(
                                wtp[:, b * P : (b + 1) * P],
                                wn[:, b * P : (b + 1) * P],
                                ident[:],
                            )
                        nc.vector.tensor_copy(
                            wpart[:, 4 * ch : 4 * ch + 4, ob * P : (ob + 1) * P],
                            wtp[:].rearrange("p (b o) -> p b o", b=4),
                        )

                # -------- allgather wT across the 4 token-shard cores --------
                if mode != 'mm':
                    wp_dram = dram_pool.tile([P, N_IT, OSH], DT, tag="wp_dram")
                    wg_dram = dram_pool.tile([R, P, N_IT, OSH], DT, tag="wg_dram")
                    nc.gpsimd.dma_start(wp_dram[:], wpart[:])
                    nc.gpsimd.collective_compute(
                        "AllGather",
                        mybir.AluOpType.bypass,
                        replica_groups=[[0, 2, 4, 6], [1, 3, 5, 7]],
                        ins=[wp_dram.opt()],
                        outs=[wg_dram.opt()],
                    )
                    for r in range(R):
                        nc.sync.dma_start(wT[:, :, r * OSH : (r + 1) * OSH], wg_dram[r])

                # ---------------- main stage ----------------
                for sl in (range(0) if mode == 'w' else range(N_SLAB)):
                    tcols = slice(sl * TSLAB, (sl + 1) * TSLAB)
                    xs = xs_pool.tile([P, N_IT, TSLAB], DT, tag="xs")
                    for it in range(N_IT):
                        nc.gpsimd.dma_start(
                            xs[:, it, :], xt_d[it * P : (it + 1) * P, tcols]
                        )  # fp32 -> bf16 cast
                    for v in range(TSLAB // P):
                        tt = sl * (TSLAB // P) + v
                        trow = slice(tt * P, (tt + 1) * P)
                        if mode == 'dma':
                            yt = yo_pool.tile([P, OC], mybir.dt.float32, tag="yt")
                            nc.vector.tensor_copy(yt[:], bias_sb[:])
                            nc.sync.dma_start(y_d[trow, :], yt[:])
                            continue
                        ps0 = mm_ps.tile([P, 512], mybir.dt.float32, tag="ps0")
                        ps1 = mm_ps.tile([P, 512], mybir.dt.float32, tag="ps1")
                        for it in range(N_IT):
                            lhsT = xs[:, it, v * P : (v + 1) * P]
                            nc.tensor.matmul(
                                ps0[:],
                                lhsT,
                                wT[:, it, 0:512],
                                start=(it == 0),
                                stop=(it == N_IT - 1),
                            )
                            nc.tensor.matmul(
                                ps1[:],
                                lhsT,
                                wT[:, it, 512:1024],
                                start=(it == 0),
                                stop=(it == N_IT - 1),
                            )
                        yt = yo_pool.tile([P, OC], mybir.dt.float32, tag="yt")
                        nc.vector.tensor_tensor(
                            yt[:, 0:512], ps0[:], bias_sb[:, 0:512], mybir.AluOpType.add
                        )
                        nc.vector.tensor_tensor(
                            yt[:, 512:1024],
                            ps1[:],
                            bias_sb[:, 512:1024],
                            mybir.AluOpType.add,
                        )
                        nc.sync.dma_start(y_d[trow, :], yt[:])

    nc.compile()
    return nc


def get_built():
    global _BUILT
    if _BUILT is None:
        _BUILT = _build_bass()
    return _BUILT


def make_in_maps(
    input, pweight, nweight, exps, bexps, mask_weight, scale, pbias, nbias, biasscale
):
    import ml_dtypes

    input = np.asarray(input, dtype=np.float32)
    pweight = np.asarray(pweight, dtype=np.float32)
    nweight = np.asarray(nweight, dtype=np.float32)
    exps = np.asarray(exps, dtype=np.float32)
    bexps = np.asarray(bexps, dtype=np.float32)
    mask_weight = np.asarray(mask_weight, dtype=np.float32)
    scale = np.asarray(scale, dtype=np.float32)
    pbias = np.asarray(pbias, dtype=np.float32)
    nbias = np.asarray(nbias, dtype=np.float32)
    biasscale = np.asarray(biasscale, dtype=np.float32)

    # tiny launch constants, computed exactly as the reference does
    mask = 1.0 / (1.0 + np.exp(-mask_weight))
    c4 = (exps * mask * scale[0]).astype(np.float32)  # [4]
    cvec = np.ascontiguousarray(
        np.broadcast_to(c4, (P, NB)).astype(ml_dtypes.bfloat16)
    )  # [128, 4]

    bias_raw = (pbias - nbias) @ bexps  # [O]
    step = float(2**NB - 1)
    b = np.clip(bias_raw, -1.0, 1.0)
    bias = (np.round(np.abs(b) * step) / step * np.sign(b)) * biasscale[0]
    bias = bias.astype(np.float32)

    x = input.reshape(T, I)
    in_maps = []
    for core in range(N_CORES):
        tr, oc = divmod(core, C)
        osl = slice(oc * OC, (oc + 1) * OC)
        wsl = slice(oc * OC + tr * OSH, oc * OC + (tr + 1) * OSH)
        in_maps.append(
            {
                "xt": np.ascontiguousarray(x[tr * TQ : (tr + 1) * TQ].T),
                "pw": pweight[wsl].reshape(OSH, IN),
                "nw": nweight[wsl].reshape(OSH, IN),
                "cvec": cvec,
                "bias": np.ascontiguousarray(np.broadcast_to(bias[osl], (P, OC))),
            }
        )
    return in_maps


def gather_output(results):
    y = np.empty((T, O), dtype=np.float32)
    for core, r in enumerate(results):
        tr, oc = divmod(core, C)
        y[tr * TQ : (tr + 1) * TQ, oc * OC : (oc + 1) * OC] = r["y"]
    return y.reshape(8, T // 8, O)


def kernel(**inputs) -> np.ndarray:
    in_maps = make_in_maps(**inputs)
    nc = get_built()
    res = run_bass_kernel_spmd(nc, in_maps, core_ids=list(range(N_CORES)))
    return gather_output(res.results)


# revision 8
# speedup vs baseline: 349.1283x; 349.1283x over previous
"""BitLinear TRN2 kernel: y = x @ W(pweight,nweight)^T + bias.

Sharding: 8 cores = 4 token-shards x 2 out-feature-shards.
Per core: xT_c [2048, 4096] (token slice, uploaded transposed as part of the
sharding layout), pw/nw [1024, 2048, 4] (out-feature slice).

Device pipeline (bf16 compute, fp32 PSUM accumulation):
  weights: DMA pw/nw fp32 -> ACT sigmoid (bf16 out) -> DVE subtract
           -> PE transpose (128x128 blocks) -> PE combine-matmul with a
           [128,32] constant C (C[4i+n, i] = exps[n]*sigmoid(mask[n])*scale)
           -> wT [i, o] bf16 in SBUF
  x:       SWDGE DMA-cast fp32->bf16 of transposed slabs -> xT [i, t] bf16
  main:    psum[t,o] += xT_tile.T @ wT_tile over 16 i-tiles; DVE adds bias
           (host-replicated [128, OC] tile) during PSUM->SBUF copy; DMA out.

bias path: bit_ste is an exact identity on the reference's bias_raw values
(k/15 grid), computed host-side along with the tiny C matrix.
"""

import numpy as np

import concourse.bass as bass
import concourse.mybir as mybir
import concourse.tile as tile
from concourse import bacc
from concourse.bass_utils import run_bass_kernel_spmd
from concourse.masks import make_identity

N_CORES = 8
R, C = 4, 2  # token shards x out-feature shards
T, I, O, NB = 16384, 2048, 2048, 4
TQ, OC = T // R, O // C  # 4096 tokens, 1024 outs per core
P = 128
IN = I * NB  # 8192 flattened (i, n) columns of pw/nw
WCH = 2048  # weight free-chunk: 512 i x 4 n
N_IT = I // P  # 16 i-tiles
N_TT = TQ // P  # 32 t-tiles
N_OB = OC // P  # 8 o-blocks
OSH = OC // R  # 256: o-rows of weight prep done locally per core
N_OBP = OSH // P  # 2 local o-blocks
N_WCH = IN // WCH  # 4 chunks per o-block
TSLAB = 512  # tokens per x slab (4 t-tiles)
N_SLAB = TQ // TSLAB
DT = mybir.dt.bfloat16

_BUILT = None


def _build_bass(reps=1, mode='full'):
    nc = bacc.Bacc("TRN2", debug=False, num_devices=N_CORES)

    xt_d = nc.dram_tensor("xt", [I, TQ], mybir.dt.float32, kind="ExternalInput").ap()
    pw_d = nc.dram_tensor("pw", [OSH, IN], mybir.dt.float32, kind="ExternalInput").ap()
    nw_d = nc.dram_tensor("nw", [OSH, IN], mybir.dt.float32, kind="ExternalInput").ap()
    cv_d = nc.dram_tensor("cvec", [P, NB], mybir.dt.bfloat16, kind="ExternalInput").ap()
    bias_d = nc.dram_tensor("bias", [P, OC], mybir.dt.float32, kind="ExternalInput").ap()
    y_d = nc.dram_tensor("y", [TQ, OC], mybir.dt.float32, kind="ExternalOutput").ap()

    with tile.TileContext(nc) as tc:
        with (
            tc.tile_pool(name="const", bufs=1) as const_pool,
            tc.tile_pool(name="wT", bufs=2) as wT_pool,
            tc.tile_pool(name="wpart", bufs=2) as wpart_pool,
            tc.tile_pool(name="dram", bufs=2, space="DRAM") as dram_pool,
            tc.tile_pool(name="wio", bufs=2) as wio_pool,
            tc.tile_pool(name="sig", bufs=2) as sig_pool,
            tc.tile_pool(name="soft", bufs=2) as soft_pool,
            tc.tile_pool(name="scl", bufs=2) as scl_pool,
            tc.tile_pool(name="wn", bufs=2) as wn_pool,
            tc.tile_pool(name="xs", bufs=2) as xs_pool,
            tc.tile_pool(name="yo", bufs=3) as yo_pool,
            tc.tile_pool(name="wtp_ps", bufs=2, space="PSUM") as wtp_ps,
            tc.tile_pool(name="mm_ps", bufs=3, space="PSUM") as mm_ps,
        ):
            ident = const_pool.tile([P, P], mybir.dt.float32)
            make_identity(nc, ident[:])
            cv_sb = const_pool.tile([P, NB], DT)
            nc.sync.dma_start(cv_sb[:], cv_d[:])
            bias_sb = const_pool.tile([P, OC], mybir.dt.float32)
            nc.sync.dma_start(bias_sb[:], bias_d[:])

            for _rep in range(reps):
                if mode != 'dma':
                    wT = wT_pool.tile([P, N_IT, OC], DT, tag="wT")
                    wpart = wpart_pool.tile([P, N_IT, OSH], DT, tag="wpart")

                # ---------------- weight stage (local o-slice) ----------------
                for ob in (range(0) if mode == 'mm' else range(N_OBP)):
                    orow = slice(ob * P, (ob + 1) * P)
                    for ch in range(N_WCH):
                        fcol = slice(ch * WCH, (ch + 1) * WCH)
                        pwt = wio_pool.tile([P, WCH], mybir.dt.float32, tag="pw")
                        nc.sync.dma_start(pwt[:], pw_d[orow, fcol])
                        nwt = wio_pool.tile([P, WCH], mybir.dt.float32, tag="nw")
                        nc.sync.dma_start(nwt[:], nw_d[orow, fcol])

                        if mode == 'dma':
                            continue
                        sp = sig_pool.tile([P, WCH], DT, tag="sp")
                        nc.scalar.activation(
                            sp[:], pwt[:], mybir.ActivationFunctionType.Sigmoid
                        )
                        sn = sig_pool.tile([P, WCH], DT, tag="sn")
                        nc.scalar.activation(
                            sn[:], nwt[:], mybir.ActivationFunctionType.Sigmoid
                        )
                        soft = soft_pool.tile([P, WCH], DT, tag="soft")
                        nc.vector.tensor_sub(out=soft[:], in0=sp[:], in1=sn[:])

                        # scaled[o, i, n] = soft * c[n]; w_nat[o, i] = sum_n
                        ICH = WCH // NB  # 512 i per chunk = i-tiles 4ch..4ch+3
                        scaled = scl_pool.tile([P, WCH], DT, tag="scl")
                        nc.vector.tensor_tensor(
                            scaled[:].rearrange("p (i n) -> p i n", n=NB),
                            soft[:].rearrange("p (i n) -> p i n", n=NB),
                            cv_sb[:, None, :].to_broadcast((P, ICH, NB)),
                            mybir.AluOpType.mult,
                        )
                        wn = wn_pool.tile([P, ICH], mybir.dt.float32, tag="wn")
                        nc.vector.tensor_reduce(
                            wn[:],
                            scaled[:].rearrange("p (i n) -> p i n", n=NB),
                            axis=mybir.AxisListType.X,
                            op=mybir.AluOpType.add,
                        )
                        # transpose w_nat [o 128, i 512] -> wT [i, o] per 128-block
                        wtp = wtp_ps.tile([P, 512], mybir.dt.float32, tag="wtp")
                        for b in range(4):
                            nc.tensor.transpose(
                                wtp[:, b * P : (b + 1) * P],
                                wn[:, b * P : (b + 1) * P],
                                ident[:],
                            )
                        nc.vector.tensor_copy(
                            wpart[:, 4 * ch : 4 * ch + 4, ob * P : (ob + 1) * P],
                            wtp[:].rearrange("p (b o) -> p b o", b=4),
                        )

                # -------- allgather wT across the 4 token-shard cores --------
                if mode not in ('mm', 'dma'):
                    wp_dram = dram_pool.tile([P, N_IT, OSH], DT, tag="wp_dram")
                    wg_dram = dram_pool.tile([R, P, N_IT, OSH], DT, tag="wg_dram")
                    nc.gpsimd.dma_start(wp_dram[:], wpart[:])
                    nc.gpsimd.collective_compute(
                        "AllGather",
                        mybir.AluOpType.bypass,
                        replica_groups=[[0, 2, 4, 6], [1, 3, 5, 7]],
                        ins=[wp_dram.opt()],
                        outs=[wg_dram.opt()],
                    )
                    for r in range(R):
                        nc.sync.dma_start(wT[:, :, r * OSH : (r + 1) * OSH], wg_dram[r])

                # ---------------- main stage ----------------
                for sl in (range(0) if mode == 'w' else range(N_SLAB)):
                    tcols = slice(sl * TSLAB, (sl + 1) * TSLAB)
                    xs = xs_pool.tile([P, N_IT, TSLAB], DT, tag="xs")
                    for it in range(N_IT):
                        nc.gpsimd.dma_start(
                            xs[:, it, :], xt_d[it * P : (it + 1) * P, tcols]
                        )  # fp32 -> bf16 cast
                    for v in range(TSLAB // P):
                        tt = sl * (TSLAB // P) + v
                        trow = slice(tt * P, (tt + 1) * P)
                        if mode == 'dma':
                            yt = yo_pool.tile([P, OC], mybir.dt.float32, tag="yt")
                            nc.vector.tensor_copy(yt[:], bias_sb[:])
                            nc.sync.dma_start(y_d[trow, :], yt[:])
                            continue
                        ps0 = mm_ps.tile([P, 512], mybir.dt.float32, tag="ps0")
                        ps1 = mm_ps.tile([P, 512], mybir.dt.float32, tag="ps1")
                        for it in range(N_IT):
                            lhsT = xs[:, it, v * P : (v + 1) * P]
                            nc.tensor.matmul(
                                ps0[:],
                                lhsT,
                                wT[:, it, 0:512],
                                start=(it == 0),
                                stop=(it == N_IT - 1),
                            )
                            nc.tensor.matmul(
                                ps1[:],
                                lhsT,
                                wT[:, it, 512:1024],
                                start=(it == 0),
                                stop=(it == N_IT - 1),
                            )
                        yt = yo_pool.tile([P, OC], mybir.dt.float32, tag="yt")
                        nc.vector.tensor_tensor(
                            yt[:, 0:512], ps0[:], bias_sb[:, 0:512], mybir.AluOpType.add
                        )
                        nc.vector.tensor_tensor(
                            yt[:, 512:1024],
                            ps1[:],
                            bias_sb[:, 512:1024],
                            mybir.AluOpType.add,
                        )
                        nc.sync.dma_start(y_d[trow, :], yt[:])

    nc.compile()
    return nc


def get_built():
    global _BUILT
    if _BUILT is None:
        _BUILT = _build_bass()
    return _BUILT


def make_in_maps(
    input, pweight, nweight, exps, bexps, mask_weight, scale, pbias, nbias, biasscale
):
    import ml_dtypes

    input = np.asarray(input, dtype=np.float32)
    pweight = np.asarray(pweight, dtype=np.float32)
    nweight = np.asarray(nweight, dtype=np.float32)
    exps = np.asarray(exps, dtype=np.float32)
    bexps = np.asarray(bexps, dtype=np.float32)
    mask_weight = np.asarray(mask_weight, dtype=np.float32)
    scale = np.asarray(scale, dtype=np.float32)
    pbias = np.asarray(pbias, dtype=np.float32)
    nbias = np.asarray(nbias, dtype=np.float32)
    biasscale = np.asarray(biasscale, dtype=np.float32)

    # tiny launch constants, computed exactly as the reference does
    mask = 1.0 / (1.0 + np.exp(-mask_weight))
    c4 = (exps * mask * scale[0]).astype(np.float32)  # [4]
    cvec = np.ascontiguousarray(
        np.broadcast_to(c4, (P, NB)).astype(ml_dtypes.bfloat16)
    )  # [128, 4]

    bias_raw = (pbias - nbias) @ bexps  # [O]
    step = float(2**NB - 1)
    b = np.clip(bias_raw, -1.0, 1.0)
    bias = (np.round(np.abs(b) * step) / step * np.sign(b)) * biasscale[0]
    bias = bias.astype(np.float32)

    x = input.reshape(T, I)
    in_maps = []
    for core in range(N_CORES):
        tr, oc = divmod(core, C)
        osl = slice(oc * OC, (oc + 1) * OC)
        wsl = slice(oc * OC + tr * OSH, oc * OC + (tr + 1) * OSH)
        in_maps.append(
            {
                "xt": np.ascontiguousarray(x[tr * TQ : (tr + 1) * TQ].T),
                "pw": pweight[wsl].reshape(OSH, IN),
                "nw": nweight[wsl].reshape(OSH, IN),
                "cvec": cvec,
                "bias": np.ascontiguousarray(np.broadcast_to(bias[osl], (P, OC))),
            }
        )
    return in_maps


def gather_output(results):
    y = np.empty((T, O), dtype=np.float32)
    for core, r in enumerate(results):
        tr, oc = divmod(core, C)
        y[tr * TQ : (tr + 1) * TQ, oc * OC : (oc + 1) * OC] = r["y"]
    return y.reshape(8, T // 8, O)


def kernel(**inputs) -> np.ndarray:
    in_maps = make_in_maps(**inputs)
    nc = get_built()
    res = run_bass_kernel_spmd(nc, in_maps, core_ids=list(range(N_CORES)))
    return gather_output(res.results)
